# revision 19
# baseline (speedup 1.0000x reference)
"""Bucket (block-diagonal) attention layer for Trainium2, 8 NeuronCores SPMD.

Sharding: data-parallel over batch (4) x tensor-parallel over head groups (2).
Core c = b*2 + g handles batch b, global heads [g*8, g*8+8).

Per-core math (local out dim 512 = 8 heads x 64):
  qT[dl, t] = sum_k Wq[g*512+dl, k] * x[b, t, k]  (+ bq)   [transposed layout]
  kT[dl, t] = likewise (bk dropped: constant-per-row score shifts cancel in
              softmax -- only bq enters scores via bq . k_j)
  v[t, dl]  = natural layout, with a ones-column appended per head so the
              attended matmul also produces the softmax denominator.
  scoresT[kt, qt] = matmul(lhsT=kT_head, rhs=qT_head)      (K=64)
  expT = exp(scoresT)  (no max subtraction; logits sigma ~3.3, safe)
  att[qt, 0:64], den[qt] = matmul(lhsT=expT, rhs=[v_head | ones])
  y = att / den + (x_slice + bv)   [residual + bv folded on host]

v2 changes vs baseline:
  - expT and v are bf16 (was f32): attended matmul runs single-pass at
    1 cycle/row instead of fp32's 4, and its LDWEIGHTS halves.
  - scores/attended matmuls for 4 heads chain into ONE psum bank via
    start=False (start=True clears the whole bank), so exp becomes one
    [128,512] activation per 4 heads and the reciprocal one strided
    [128,4] op per 4 heads -- amortizing per-instruction overhead.

All matmuls f32-accumulate in PSUM; softmax/normalize in f32.
"""

import json
import sys

import numpy as np
import ml_dtypes

BF16 = ml_dtypes.bfloat16
FP16 = np.float16

B, S, D = 4, 4096, 1024
H, NB = 16, 32
HG = 2            # head groups (tensor parallel over heads)
NCORES = B * HG   # 8
DL = D // HG      # 512 local output dims per core
HL = H // HG      # 8 local heads
HD = D // H       # 64 head dim
BS = S // NB      # 128 bucket size
KC = D // 128     # 8 contraction chunks
NQ = 4            # token quarters processed as pipeline phases
TOKQ = S // NQ    # 1024 tokens per quarter
NBQ = TOKQ // BS  # 8 buckets per quarter
VW = 66           # per-head block width in v tiles: 64 data + 1 ones + 1 pad

_built = None     # cached (nc,) so repeated kernel() calls reuse the program


def _apply_waitfix():
    """This container's walrus accepts at most ONE sem wait per instruction.
    Post-process the BIR json: hoist extra waits onto injected wait-only
    EventSemaphore instructions just before the owning instruction."""
    import concourse.bass as bass

    if getattr(bass.Bass, "_waitfix_applied", False):
        return
    orig = bass.Bass.to_json_bytes

    def _split(m):
        n = 0
        for f in m["functions"]:
            for blk in f["blocks"]:
                out = []
                for inst in blk["instructions"]:
                    si = inst.get("sync_info")
                    if si and si.get("on_wait") and len(si["on_wait"]) > 1:
                        waits = si["on_wait"]
                        si["on_wait"] = waits[-1:]
                        for k, w in enumerate(waits[:-1]):
                            out.append({
                                "debug": inst.get("debug", 0),
                                "engine": inst["engine"],
                                "ins": [],
                                "outs": [],
                                "name": f"wfix{n}_{k}_{inst['name']}",
                                "opcode": "EventSemaphore",
                                "sync_info": {"on_update": [], "on_wait": [w]},
                            })
                        n += 1
                    out.append(inst)
                blk["instructions"] = out
        return n

    def patched(self):
        m = json.loads(orig(self))
        _split(m)
        return json.dumps(m).encode()

    bass.Bass.to_json_bytes = patched
    bass.Bass._waitfix_applied = True


def _build():
    global _built
    if _built is not None:
        return _built

    _apply_waitfix()
    import concourse.bass as bass
    import concourse.tile as tile
    from concourse import mybir
    from concourse.bass import ts

    f32 = mybir.dt.float32
    f16 = mybir.dt.float16
    bf16 = mybir.dt.bfloat16
    Act = mybir.ActivationFunctionType
    Alu = mybir.AluOpType

    nc = bass.Bass()
    # xt pre-tiled on host: [kchunk, half-quarter, 128, 512] so each SBUF
    # tile load is one fully-contiguous 128KB DMA.
    xt = nc.dram_tensor("xt", [KC, NQ * 2, 128, 512], f16,
                        kind="ExternalInput")
    wq = nc.dram_tensor("wq", [D, DL], f16, kind="ExternalInput")
    wk = nc.dram_tensor("wk", [D, DL], f16, kind="ExternalInput")
    wv = nc.dram_tensor("wv", [D, DL], f16, kind="ExternalInput")
    bqt = nc.dram_tensor("bq", [128, DL // 128], f32, kind="ExternalInput")
    xres = nc.dram_tensor("xres", [S, DL], f32, kind="ExternalInput")
    y = nc.dram_tensor("y", [S, DL], f32, kind="ExternalOutput")

    OD = DL // 128  # 4 out-dim partition tiles for qT/kT

    with tile.TileContext(nc) as tc:
        with (
            tc.tile_pool(name="wpool", bufs=1) as wpool,
            tc.tile_pool(name="xtp", bufs=32) as xtp,
            tc.tile_pool(name="qtp", bufs=2 * OD) as qtp,
            tc.tile_pool(name="ktp", bufs=2 * OD) as ktp,
            tc.tile_pool(name="vp", bufs=2 * NBQ) as vpool,
            tc.tile_pool(name="ep", bufs=4) as epool,
            tc.tile_pool(name="yp", bufs=3) as ypool,
            tc.tile_pool(name="xrp", bufs=4) as xrpool,
            tc.tile_pool(name="rp", bufs=8) as rpool,
            tc.tile_pool(name="ps_qkv", bufs=2, space="PSUM") as ps_qkv,
            tc.tile_pool(name="ps_s", bufs=3, space="PSUM") as ps_s,
            tc.tile_pool(name="ps_a", bufs=3, space="PSUM") as ps_a,
        ):
            # --- stationary weights + bias, loaded once ---
            # wq + quarter-0 xT first: they alone gate the first projection
            # group, pulling the first matmul ~10us earlier.
            def wload(lst, src, nm):
                for kk in range(KC):
                    t = wpool.tile([128, DL], f16, tag=f"{nm}{kk}",
                                   name=f"{nm}{kk}")
                    nc.sync.dma_start(out=t[:], in_=src[ts(kk, 128), :])
                    lst.append(t)

            def xload(q):
                # half-quarter granularity: the first 512-token projection
                # group only waits on 8 x 128KB loads
                tiles = [[None] * 2 for _ in range(KC)]
                for tt in range(2):
                    for kk in range(KC):
                        t = xtp.tile([128, 512], f16, tag="xt")
                        nc.sync.dma_start(out=t[:],
                                          in_=xt[kk, q * 2 + tt, :, :])
                        tiles[kk][tt] = t
                return tiles

            wq_sb, wk_sb, wv_sb = [], [], []
            wload(wq_sb, wq, "wq")
            bq_sb = wpool.tile([128, OD], f32, tag="bq")
            nc.sync.dma_start(out=bq_sb[:], in_=bqt[:, :])
            xt0_sb = xload(0)
            wload(wk_sb, wk, "wk")
            wload(wv_sb, wv, "wv")

            for q in range(NQ):
                tok0 = q * TOKQ
                xt_sb = xt0_sb if q == 0 else xload(q)

                # --- q/k projections: psum[od-tile, 512 tok] over 8 k-chunks
                qt_sb = [qtp.tile([128, TOKQ], f16, tag="qt", name="qt")
                         for _ in range(OD)]
                kt_sb = [ktp.tile([128, TOKQ], f16, tag="kt", name="kt")
                         for _ in range(OD)]
                for od in range(OD):
                    for tt in range(TOKQ // 512):
                        pq = ps_qkv.tile([128, 512], f32, tag="pqkv")
                        for kk in range(KC):
                            nc.tensor.matmul(
                                pq[:], wq_sb[kk][:, ts(od, 128)],
                                xt_sb[kk][tt][:],
                                start=(kk == 0), stop=(kk == KC - 1))
                        nc.scalar.activation(
                            qt_sb[od][:, ts(tt, 512)], pq[:], Act.Identity,
                            bias=bq_sb[:, od:od + 1], scale=1.0)
                        pk = ps_qkv.tile([128, 512], f32, tag="pqkv")
                        for kk in range(KC):
                            nc.tensor.matmul(
                                pk[:], wk_sb[kk][:, ts(od, 128)],
                                xt_sb[kk][tt][:],
                                start=(kk == 0), stop=(kk == KC - 1))
                        nc.scalar.copy(kt_sb[od][:, ts(tt, 512)], pk[:])

                # --- v projection (natural layout), one bucket per psum ---
                v_sb = []
                for vt in range(NBQ):
                    pv = ps_qkv.tile([128, 512], f32, tag="pqkv")
                    for kk in range(KC):
                        nc.tensor.matmul(
                            pv[:],
                            xt_sb[kk][vt // 4][:, ts(vt % 4, 128)],
                            wv_sb[kk][:],
                            start=(kk == 0), stop=(kk == KC - 1))
                    vt_sb = vpool.tile([128, HL * VW], bf16, tag="v")
                    v3 = vt_sb[:].rearrange("p (h c) -> p h c", c=VW)
                    nc.vector.memset(v3[:, :, 64:66], 1.0)
                    nc.vector.tensor_copy(
                        v3[:, :, 0:64],
                        pv[:].rearrange("p (h c) -> p h c", c=HD))
                    v_sb.append(vt_sb)

                # --- attention per bucket ---
                # Head h sits at od-tile h%4, partitions (h//4)*64.. (host
                # permutes Wq/Wk rows to match), so each bank's 4 chained
                # score MMs share one base partition -- mixed base
                # partitions in one psum bank crash the PE. The two score
                # chains interleave so consecutive MMs hit disjoint 64-row
                # groups (po=0 vs po=64) and overlap in the array; both are
                # emitted before the attended groups so exp(g0) overlaps
                # scores(g1) instead of stalling the PE.

                def emit_scores(bk, pool):
                    col = ts(bk, BS)
                    tg = "ps" if pool is ps_s else "pqkv"
                    psc_g = [pool.tile([128, 512], f32, tag=tg, name=tg)
                             for _ in range(2)]
                    for hh in range(4):
                        for g in range(2):
                            po = g * 64
                            nc.tensor.matmul(
                                psc_g[g][:, ts(hh, 128)],
                                kt_sb[hh][po:po + 64, col],
                                qt_sb[hh][po:po + 64, col],
                                start=(hh == 0), stop=(hh == 3),
                                skip_group_check=True)
                    ex_g = []
                    for g in range(2):
                        ex = epool.tile([128, 512], bf16, tag="ex",
                                        name="ex")
                        nc.scalar.activation(ex[:], psc_g[g][:], Act.Exp)
                        ex_g.append(ex)
                    return ex_g

                def emit_attended(bk, ex_g, xr, yt):
                    for g in range(2):
                        ex = ex_g[g]
                        pa = ps_a.tile([128, 512], f32, tag="pa", name="pa")
                        for hh in range(4):
                            h = g * 4 + hh
                            nc.tensor.matmul(
                                pa[:, hh * 128:hh * 128 + 66],
                                ex[:, ts(hh, 128)],
                                v_sb[bk][:, h * VW:(h + 1) * VW],
                                start=(hh == 0), stop=(hh == 3),
                                skip_group_check=True)
                        rc = rpool.tile([128, 4], f32, tag="r", name="rc")
                        pa3 = pa[:].rearrange("p (h c) -> p h c", c=128)
                        nc.vector.reciprocal(rc[:], pa3[:, :, 64])
                        for hh in range(4):
                            h = g * 4 + hh
                            nc.vector.scalar_tensor_tensor(
                                out=yt[:, ts(h, HD)],
                                in0=pa[:, hh * 128:hh * 128 + 64],
                                scalar=rc[:, hh:hh + 1],
                                in1=xr[:, ts(h, HD)],
                                op0=Alu.mult, op1=Alu.add)

                def load_xr(bk):
                    # gpsimd queue: buffer-reuse waits must not block xt
                    # loads behind them on the sync queue (vector cannot
                    # issue DMAs).
                    xr = xrpool.tile([128, DL], f32, tag="xres")
                    nc.gpsimd.dma_start(
                        out=xr[:],
                        in_=xres[tok0 + bk * BS:tok0 + (bk + 1) * BS, :])
                    return xr

                def store_y(bk, yt):
                    nc.gpsimd.dma_start(
                        out=y[tok0 + bk * BS:tok0 + (bk + 1) * BS, :],
                        in_=yt[:])

                if q < NQ - 1:
                    for bk in range(NBQ):
                        xr = load_xr(bk)
                        yt = ypool.tile([128, DL], f32, tag="yt")
                        ex_g = emit_scores(bk, ps_s)
                        emit_attended(bk, ex_g, xr, yt)
                        store_y(bk, yt)
                else:
                    # Last quarter has no projection matmuls left to hide
                    # the exp latency: process buckets in pairs, the odd
                    # bucket's scores borrowing the now-idle ps_qkv banks,
                    # so exp(b+1) overlaps attended(b).
                    for b0 in range(0, NBQ, 2):
                        b1 = b0 + 1
                        xr0, xr1 = load_xr(b0), load_xr(b1)
                        yt0 = ypool.tile([128, DL], f32, tag="yt")
                        yt1 = ypool.tile([128, DL], f32, tag="yt")
                        ex0 = emit_scores(b0, ps_s)
                        ex1 = emit_scores(b1, ps_qkv)
                        emit_attended(b0, ex0, xr0, yt0)
                        emit_attended(b1, ex1, xr1, yt1)
                        store_y(b0, yt0)
                        store_y(b1, yt1)

    _built = nc
    return nc


def _prep_in_maps(x, Wq, bq, Wk, bk, Wv, bv):
    x = np.asarray(x, np.float32)
    Wq = np.asarray(Wq, np.float32)
    Wv = np.asarray(Wv, np.float32)
    Wk = np.asarray(Wk, np.float32)
    bq = np.asarray(bq, np.float32)
    bv = np.asarray(bv, np.float32)

    # [D, S] -> [KC, NQ*2, 128, 512] pre-tiled so each SBUF tile load is
    # one contiguous DMA
    xt_b = [np.ascontiguousarray(
        x[b].T.reshape(KC, 128, NQ * 2, 512).transpose(0, 2, 1, 3)
    ).astype(FP16) for b in range(B)]
    # qT/kT row permutation: head h -> od-tile h%4, partitions (h//4)*64..
    # so score banks group 4 heads sharing one base partition.
    perm = np.empty(DL, np.int64)
    for h in range(HL):
        for i in range(HD):
            perm[(h % 4) * 128 + (h // 4) * 64 + i] = h * HD + i
    wq_g, wk_g, wv_g, bq_g = [], [], [], []
    for g in range(HG):
        sl = slice(g * DL, (g + 1) * DL)
        wq_g.append(np.ascontiguousarray(Wq[sl, :][perm].T).astype(FP16))
        wk_g.append(np.ascontiguousarray(Wk[sl, :][perm].T).astype(FP16))
        wv_g.append(np.ascontiguousarray(Wv[sl, :].T).astype(FP16))
        bq_g.append(np.ascontiguousarray(
            bq[sl][perm].reshape(DL // 128, 128).T).astype(np.float32))

    in_maps = []
    for c in range(NCORES):
        b, g = c // HG, c % HG
        sl = slice(g * DL, (g + 1) * DL)
        xres = (x[b][:, sl] + bv[None, sl]).astype(np.float32)
        in_maps.append({
            "xt": xt_b[b], "wq": wq_g[g], "wk": wk_g[g], "wv": wv_g[g],
            "bq": bq_g[g], "xres": np.ascontiguousarray(xres),
        })
    return in_maps


def _gather(results):
    out = np.empty((B, S, D), np.float32)
    for c, r in enumerate(results):
        b, g = c // HG, c % HG
        out[b, :, g * DL:(g + 1) * DL] = r["y"]
    return out


def _run(inputs, trace=False, trace_cores=None):
    nc = _build()
    from concourse.bass_utils import run_bass_kernel_spmd

    in_maps = _prep_in_maps(**inputs)
    res = run_bass_kernel_spmd(
        nc, in_maps, core_ids=list(range(NCORES)), trace=trace,
        trace_cores=trace_cores)
    return _gather(res.results), res


def kernel(**inputs):
    out, _ = _run(inputs, trace=False)
    return out


def kernel_traced(trace_cores=None, **inputs):
    """For test.py: returns (output, BassKernelResults with exec_time_ns)."""
    import types
    import trn_agent_boot.trn_boot as tb

    if "antenv.axon_hooks" not in sys.modules:
        hooks = types.ModuleType("antenv.axon_hooks")
        state = [None]
        hooks.set_axon_ntff_profile_hook = lambda h: state.__setitem__(0, h)
        hooks.get_axon_ntff_profile_hook = lambda: state[0]
        sys.modules["antenv.axon_hooks"] = hooks
        hooks.set_axon_ntff_profile_hook(
            tb._ntff_profile_via_ctypes("/opt/axon/libaxon_pjrt.so"))
    return _run(inputs, trace=True, trace_cores=trace_cores)
